# revision 39
# baseline (speedup 1.0000x reference)
"""MoE expert MLP (SwiGLU, top-2 routing) on 8 Trainium2 NeuronCores.

Strategy: expert-parallel. Host routes tokens (stable argsort by expert id,
matching the reference), gathers each expert's token rows, and pads them to a
fixed capacity C. Core e runs expert e's two GEMMs + SwiGLU over its C-column
token panel; the host scatters results back into the permuted [N, H] output.

v2 dataflow — fully fused pair pipeline. The kernel is DMA-bound (12.9MB of
weights per core at ~360GB/s ≈ 36us), so the schedule is built so the PE
consumes each weight byte as it lands and the program ends right behind the
last DMA byte:

  stream (one HWDGE ring, exact consumption order):
     [xt | w1 pair0] , [w1 pair1 | w2 row0] , ... , [w1 pair15 | w2 row14],
     [w2 row15]
  per pair j: pa/pb = w1(a_j/b_j)^T @ xt (2x8 matmuls, PSUM chains)
              it_j  = silu(pa)*pb  (ACT then DVE, bf16)
              yacc[h] += w2[ki=j, h]^T @ it_j  for all 8 h-blocks (8 matmuls,
              PSUM accumulation chains held across ALL 16 pairs)
  The second GEMM therefore finishes ~1us after the last weight chunk lands,
  instead of waiting for the whole intermediate like a phase-split schedule.

  A pre-Tile warmup (dummy matmuls on a zeroed scratch) keeps the PE busy from
  t=0 so its p-state ramp (0.65/1.2GHz -> 2.4GHz after ~3us busy) burns during
  the initial DMA wait, not during real work.

Weights are re-laid-out on the host so each DMA lands 128-partition tiles with
multi-KB contiguous per-partition rows in exact consumption order.
"""

import numpy as np
import ml_dtypes

import concourse.bass as bass
import concourse.mybir as mybir
import concourse.tile as tile
from concourse import bacc
from concourse.bass_utils import run_bass_kernel_spmd

BF16 = mybir.dt.bfloat16
F32 = mybir.dt.float32
NP_BF16 = ml_dtypes.bfloat16

# Problem shape (hardcoded per the contract; matches nn_Experts_41429254537622)
B, S, H, I, E, TOPK = 1, 512, 1024, 2048, 8, 2
N_CORES = 8
C = 152          # per-expert token capacity per wave (max observed count 152)
KH = H // 128    # 8  k-tiles for GEMM1 (contraction over H)
NP = I // 128    # 16 (a, b) pairs of 128-wide w1 column blocks == w2 k-tiles
MH = H // 128    # 8  output row blocks of yT
PAIR = 2 * KH * 128   # 2048 cols: one w1 pair block (a_j 8 ktiles, b_j 8)
W2R = MH * 128        # 1024 cols: one w2 row block (ktile j for all 8 h)
# wall = [b0 | pair1 w2r0 | pair2 w2r1 | ... | pair15 w2r14 | w2r15]
WALL = KH * 128 + (NP - 1) * (PAIR + W2R) + W2R
N_WARM = 14      # PE p-state warmup matmuls (512 moving rows each)
HALF = KH * 128  # 1024 cols: half a w1 pair block (the a_j or b_j ktiles)
ACT_FUNC = mybir.ActivationFunctionType.Silu  # sim_check swaps (no Silu in sim)

_compiled = {}
LAST_RUNS = []  # BassKernelResults of the most recent kernel() call (for test harness)


def _build_program():
    nc = bacc.Bacc(
        "TRN2", target_bir_lowering=False, debug=False, num_devices=N_CORES
    )
    head_d = nc.dram_tensor(
        "headr", [128, KH * C + HALF], BF16, kind="ExternalInput"
    )
    wall_d = nc.dram_tensor("wallr", [128, WALL], BF16, kind="ExternalInput")
    yT_d = nc.dram_tensor("yT", [128, MH * C], BF16, kind="ExternalOutput")

    # --- pre-Tile raw section -------------------------------------------
    # PE warmup: dummy matmuls on zeroed scratch so the tensor engine's
    # p-state ramp overlaps the initial weight DMA instead of real matmuls.
    # The warmup PSUM bank is freed (stack pop) before the TileContext, so
    # the pools can use all 8 banks; all warmup writes strictly precede any
    # Tile matmul in PE program order.
    wscr = nc.alloc_sbuf_tensor("wscr", [128, 512], BF16)
    wsem = nc.alloc_semaphore(name="warm_sem")
    nc.gpsimd.memset(wscr.ap()[:, :], 0).then_inc(wsem, 1)
    nc.tensor.wait_ge(wsem, 1)
    with nc.psum_tensor("wpsum", [128, 512], F32) as wpsum:
        for _ in range(N_WARM):
            nc.tensor.matmul(
                wpsum.ap()[:, :], wscr.ap()[:, :128], wscr.ap()[:, :512],
                start=True, stop=True,
            )

    # Head pre-load (xt + w1 pair0's a-half) as a raw DMA on the sync ring,
    # streaming during the Tile framework preamble. pair0's b-half rides the
    # first Tile chunk so pa0 can start as soon as [xt|a0] lands. The wait
    # sits on the consumer (tensor queue), program-order ahead of every
    # Tile-emitted PE instruction.
    head_sb = nc.alloc_sbuf_tensor("head_sb", [128, KH * C + HALF], BF16)
    pre_sem = nc.alloc_semaphore(name="pre_dma_sem")
    nc.sync.dma_start(head_sb.ap()[:, :], head_d[:]).then_inc(pre_sem, 16)
    nc.tensor.wait_ge(pre_sem, 16)
    nc.tensor.sem_clear(pre_sem)
    xt = head_sb.ap()[:, : KH * C]
    w1a0 = head_sb.ap()[:, KH * C:]

    with tile.TileContext(nc) as tc:
        with (
            tc.tile_pool(name="wp", bufs=1) as wp,
            tc.tile_pool(name="sap", bufs=4) as sap,
            tc.tile_pool(name="itp", bufs=4) as itp,
            tc.tile_pool(name="outp", bufs=2) as outp,
            tc.tile_pool(name="ps1", bufs=5, space="PSUM") as ps1,
            tc.tile_pool(name="yp", bufs=1, space="PSUM") as yp,
        ):
            # 8 long-lived GEMM2 accumulation chains packed 3-per-PSUM-bank.
            yacc = [
                yp.tile([128, 3 * C], F32, tag=f"y{i}", name=f"yacc{i}")
                for i in range(3)
            ]

            def ydst(h):
                return yacc[h // 3][:, (h % 3) * C:(h % 3 + 1) * C]

            prev_it = None
            for j in range(NP):
                if j == 0:
                    # a0 came with the raw head; b0 is the first Tile chunk.
                    wa = w1a0
                    wb = wp.tile([128, HALF], BF16, tag="w0", bufs=1,
                                 name="wb0")
                    nc.sync.dma_start(wb[:], wall_d[:, :HALF])
                    w2prev = None
                else:
                    # One combined transfer per pair [w1 pair_j | w2 row_{j-1}]
                    # in exact consumption order on the single sync ring.
                    off = HALF + (j - 1) * (PAIR + W2R)
                    wt = wp.tile([128, PAIR + W2R], BF16, tag="w",
                                 bufs=NP - 1, name=f"w{j}")
                    nc.sync.dma_start(wt[:], wall_d[:, off: off + PAIR + W2R])
                    wa = wt[:, :HALF]
                    wb = wt[:, HALF:PAIR]
                    w2prev = wt[:, PAIR:]
                pa = ps1.tile([128, C], F32, tag="p")
                pb = ps1.tile([128, C], F32, tag="p")
                for k in range(KH):
                    nc.tensor.matmul(
                        pa[:],
                        wa[:, k * 128:(k + 1) * 128],
                        xt[:, k * C:(k + 1) * C],
                        start=(k == 0),
                        stop=(k == KH - 1),
                    )
                for k in range(KH):
                    nc.tensor.matmul(
                        pb[:],
                        wb[:, k * 128:(k + 1) * 128],
                        xt[:, k * C:(k + 1) * C],
                        start=(k == 0),
                        stop=(k == KH - 1),
                    )
                # GEMM2 contribution of the PREVIOUS pair, which consumes the
                # w2 row block that rides in this pair's DMA chunk. PSUM
                # start_tensor_calc zeroes the whole 2KB bank (zero region),
                # so the 3 chains sharing a bank form one bank-level group:
                # start only on the bank's first-ever contribution.
                if j > 0:
                    for h in range(MH):
                        nc.tensor.matmul(
                            ydst(h),
                            w2prev[:, h * 128:(h + 1) * 128],
                            prev_it[:],
                            start=(j - 1 == 0 and h % 3 == 0),
                            stop=False,
                        )
                sa = sap.tile([128, C], F32, tag="sa")
                nc.scalar.activation(sa[:], pa[:], ACT_FUNC)
                it = itp.tile([128, C], BF16, tag="it")
                nc.vector.tensor_mul(it[:], sa[:], pb[:])
                prev_it = it

            # Final w2 row (ktile 15) arrives as its own small tail chunk.
            wl = wp.tile([128, W2R], BF16, tag="wl", bufs=1)
            nc.sync.dma_start(
                wl[:], wall_d[:, HALF + (NP - 1) * (PAIR + W2R):]
            )
            for h in range(MH):
                nc.tensor.matmul(
                    ydst(h),
                    wl[:, h * 128:(h + 1) * 128],
                    prev_it[:],
                    start=False,
                    stop=(h % 3 == 2 or h == MH - 1),  # bank's last chain
                )

            # Drain the 8 accumulators: f32 PSUM -> bf16 SBUF on both ACT and
            # DVE, stored as two halves so store0 overlaps the h4-7 casts.
            for half in range(2):
                yt = outp.tile([128, 4 * C], BF16, tag="yt")
                for hh in range(4):
                    h = half * 4 + hh
                    dst = yt[:, hh * C:(hh + 1) * C]
                    if hh % 2 == 0:
                        nc.vector.tensor_copy(dst, ydst(h))
                    else:
                        nc.scalar.activation(
                            dst, ydst(h), mybir.ActivationFunctionType.Copy
                        )
                # Stores ride the same sync ring: they only become ready after
                # the last weight chunk anyway, and a single HWDGE ring avoids
                # fair-share round-robin splitting the weight stream.
                nc.sync.dma_start(
                    yT_d[:, half * 4 * C:(half + 1) * 4 * C], yt[:]
                )
    nc.compile()
    return nc


def _get_program():
    if "nc" not in _compiled:
        _compiled["nc"] = _build_program()
    return _compiled["nc"]


def _relayout_weights(w1_e, w2_e):
    """Build the head (w1 pair0 part) and wall DRAM images for one expert.

    w1_e: [H, 2I] bf16; w2_e: [I, H] bf16.
    pair_j block (2048 cols): a_j ktiles k=0..7 then b_j ktiles, each a
      [K=128, M=128] stationary tile (partition = H ktile row).
    w2row_j block (1024 cols): h=0..7 stationary tiles of w2[ki=j block, h].
    wall = [pair1 | w2row0 | pair2 | w2row1 | ... | pair15 | w2row14 | w2row15]
    """
    A = w1_e[:, :I].reshape(KH, 128, NP, 128)       # [k, p, j, m]
    Bh = w1_e[:, I:].reshape(KH, 128, NP, 128)
    Aa = A.transpose(1, 2, 0, 3)                     # [p, j, k, m]
    Ab = Bh.transpose(1, 2, 0, 3)
    pairs = np.concatenate(
        [Aa.reshape(128, NP, KH * 128), Ab.reshape(128, NP, KH * 128)], axis=2
    )                                                # [p, j, 2048]
    w2r = w2_e.reshape(NP, 128, MH, 128).transpose(1, 0, 2, 3).reshape(
        128, NP, W2R
    )                                                # [p, j, 1024]
    wall = np.empty((128, WALL), dtype=w1_e.dtype)
    wall[:, :HALF] = pairs[:, 0, HALF:]              # b0
    for j in range(1, NP):
        off = HALF + (j - 1) * (PAIR + W2R)
        wall[:, off: off + PAIR] = pairs[:, j]
        wall[:, off + PAIR: off + PAIR + W2R] = w2r[:, j - 1]
    wall[:, HALF + (NP - 1) * (PAIR + W2R):] = w2r[:, NP - 1]
    return np.ascontiguousarray(pairs[:, 0, :HALF]), np.ascontiguousarray(wall)


def kernel(hidden_states, tokens_per_expert, w1, w2):
    x = np.asarray(hidden_states).reshape(-1, H)
    flat = np.asarray(tokens_per_expert).reshape(-1).astype(np.int64)
    w1 = np.asarray(w1)
    w2 = np.asarray(w2)
    n_rows = flat.shape[0]

    order = np.argsort(flat, kind="stable")
    token_of_row = order // TOPK
    counts = np.bincount(flat, minlength=E)
    starts = np.concatenate([[0], np.cumsum(counts)[:-1]])

    x_bf = x.astype(NP_BF16)
    if w1.dtype != NP_BF16:
        w1 = w1.astype(NP_BF16)
    if w2.dtype != NP_BF16:
        w2 = w2.astype(NP_BF16)

    nc = _get_program()
    relayed = [_relayout_weights(w1[e], w2[e]) for e in range(E)]

    out = np.zeros((n_rows, H), dtype=NP_BF16)
    LAST_RUNS.clear()
    n_waves = int(max(1, -(-int(counts.max()) // C)))
    for wave in range(n_waves):
        in_maps = []
        for e in range(E):
            lo = starts[e] + wave * C
            cnt = int(min(C, max(0, counts[e] - wave * C)))
            xe = np.zeros((C, H), dtype=NP_BF16)
            if cnt:
                xe[:cnt] = x_bf[token_of_row[lo:lo + cnt]]
            # xT layout: [128, KH*C], k-tile k at cols [k*C, (k+1)*C):
            # xT[p, k*C + c] = xe[c, k*128 + p]
            xT = np.ascontiguousarray(
                xe.T.reshape(KH, 128, C).transpose(1, 0, 2).reshape(128, KH * C)
            )
            a0, wall = relayed[e]
            head = np.ascontiguousarray(np.concatenate([xT, a0], axis=1))
            in_maps.append({"headr": head, "wallr": wall})

        res = run_bass_kernel_spmd(nc, in_maps, list(range(N_CORES)))
        LAST_RUNS.append(res)
        for e in range(E):
            lo = starts[e] + wave * C
            cnt = int(min(C, max(0, counts[e] - wave * C)))
            if not cnt:
                continue
            yT = res.results[e]["yT"]
            # yT[p, h*C + c] = y[c, h*128 + p]
            y = yT.reshape(128, MH, C).transpose(2, 1, 0).reshape(C, H)
            out[lo:lo + cnt] = y[:cnt]
    return out
